# revision 62
# baseline (speedup 1.0000x reference)
"""Trainium2 Bass kernel for GQA attention (nn_Attention_15350213116218).

B=1, S=2048, D=2048, 32 q-heads / 8 kv-heads, head_dim 64, RoPE, causal, fp32.

Sharding: tensor-parallel over heads across 8 NeuronCores. Core c gets q-heads
[4c, 4c+4) and kv-head c (wq/wk/wv column-shard, wo row-shard). Each core
computes its partial output through its wo rows; the host sums the 8 partials.

Per-core device algorithm (matmul operands bf16 — enables fast weight load and
LDWEIGHTS pull-ahead on the PE; all accumulation fp32 in PSUM, softmax fp32):
  - x/weights staged host-side in exact sbuf layouts (contiguous DMAs, no
    strided-descriptor storms).
  - Q/K/V projections computed feature-major with host-permuted weight columns
    so RoPE even/odd dims land in separate partition blocks; RoPE fp32 on the
    psum, written bf16, DMA-interleaved into per-pair [h_r; h_i] tiles so
    score matmuls contract K=64, two heads packed via tile_position.
  - All persistent activations are split into per-512-chunk tiles so Tile's
    per-tile semaphores don't serialize across phases.
  - Causal softmax without max-subtraction: upper blocks skipped, score/exp/PV
    column-trimmed on diagonal superblocks, and the triangular -1e30 mask on
    diagonal blocks is accumulated INTO the score psum by the PE itself
    (identity @ maskT matmul, start=False) — the score->exp->PV chain never
    touches the DVE, so it can't queue behind rope/normalize work.
  - P@V via lhsT = [v | ones] accumulating the softmax denominator,
    software-pipelined one block behind the scores so the ACT exp round-trip
    hides behind the next score matmul; normalize: DVE psum copies,
    reciprocal_approx_fast (partition-0 staging), gpsimd partition broadcast
    (ucode pre-warmed mid-phase-A: the first broadcast after DMA work pays a
    ~7us library reload), DVE mul; the last s-chunk's projection psums are
    bounced to SBUF by the ACT so psum pools release without waiting on the
    DVE rope backlog.
  - out_proj is software-pipelined one q-superblock behind attention, with
    both wo-half matmuls of adjacent tiles paired so the second half can
    issue while the first tile's normalize completes.
"""
import math
import os
import sys

import numpy as np
import ml_dtypes

try:
    import concourse.bass as bass
except ImportError:
    sys.path.insert(0, "/opt/trn_rl_repo")
    import concourse.bass as bass

import concourse.mybir as mybir
import concourse.tile as tile
import concourse.bass_utils as bass_utils
from concourse import bacc
from concourse.masks import make_identity, make_lower_triangular

f32 = mybir.dt.float32
bf16 = mybir.dt.bfloat16
BF16 = ml_dtypes.bfloat16

S = 2048
D = 2048
NH, NKV, HD = 32, 8, 64
NCORES = 8
HPC = NH // NCORES          # 4 q heads per core
D2 = HD // 2                # 32
P = 128
SCH = 512                   # s-chunk for projections == q superblock
QSB = 512
NSCH = S // SCH             # 4
NQSB = S // QSB             # 4
NDBLK = D // P              # 16
NSBLK = S // P              # 16
SCALE = 1.0 / math.sqrt(HD)


def _build_kernel(reps=1, phases="ABEPNC"):
    nc = bacc.Bacc("TRN2", target_bir_lowering=False)

    # all staged host-side in sbuf layout (partition-major, contiguous rows;
    # x additionally s-chunk-major so each xt4 DMA descriptor is 4KB)
    xt_d = nc.dram_tensor("xT", [P, NSCH, NDBLK, SCH], bf16,
                          kind="ExternalInput").ap()
    wqr_d = nc.dram_tensor("wq_r", [P, NDBLK, P], bf16, kind="ExternalInput").ap()
    wqi_d = nc.dram_tensor("wq_i", [P, NDBLK, P], bf16, kind="ExternalInput").ap()
    wkvi_d = nc.dram_tensor("wkvi", [P, NDBLK, P], bf16, kind="ExternalInput").ap()
    wo_d = nc.dram_tensor("wo_c", [P, 2, D], bf16, kind="ExternalInput").ap()
    cos_d = nc.dram_tensor("cosT4", [P, S], f32, kind="ExternalInput").ap()
    sin_d = nc.dram_tensor("sinT4", [P, S], f32, kind="ExternalInput").ap()
    out_d = nc.dram_tensor("out", [S, D], bf16, kind="ExternalOutput").ap()

    with tile.TileContext(nc) as tc:
        for r in range(reps):
            _body(tc, xt_d, wqr_d, wqi_d, wkvi_d, wo_d, cos_d, sin_d, out_d,
                  pfx=f"r{r}_" if reps > 1 else "", phases=phases)
    nc.compile()
    return nc


def _body(tc, xt_d, wqr_d, wqi_d, wkvi_d, wo_d, cos_d, sin_d, out_d, pfx="",
          phases="ABEPNC"):
    nc = tc.nc
    Exp = mybir.ActivationFunctionType.Exp

    with (
        tc.tile_pool(name=pfx + "consts", bufs=1) as consts,
        tc.tile_pool(name=pfx + "persist", bufs=1) as persist,
    ):
        _body_inner(tc, nc, Exp, consts, persist, xt_d, wqr_d, wqi_d, wkvi_d,
                    wo_d, cos_d, sin_d, out_d, pfx, phases)


def _body_inner(tc, nc, Exp, consts, persist, xt_d, wqr_d, wqi_d, wkvi_d,
                wo_d, cos_d, sin_d, out_d, pfx, phases="ABEPNC"):
    # ---- projection weights first: phase A stalls on these ----
    wq_r = consts.tile([P, NDBLK, P], bf16, tag="wq_r")
    nc.sync.dma_start(wq_r[:], wqr_d[:])
    wq_i = consts.tile([P, NDBLK, P], bf16, tag="wq_i")
    nc.scalar.dma_start(wq_i[:], wqi_d[:])
    wkvi = consts.tile([P, NDBLK, P], bf16, tag="wkvi")
    nc.gpsimd.dma_start(wkvi[:], wkvi_d[:])
    # rope tables / wo loaded inside phase A to keep x first in queue order
    cosT4 = consts.tile([P, S], f32, tag="cosT4")
    sinT4 = consts.tile([P, S], f32, tag="sinT4")
    wo_sb = consts.tile([P, 2, D], bf16, tag="wo_sb")

    # ---- constants ----
    ident = consts.tile([P, P], bf16, tag="ident")
    ident32 = consts.tile([P, P], f32, tag="ident32")
    make_identity(nc, ident32[:])
    nc.vector.tensor_copy(ident[:], ident32[:])
    maskT32 = consts.tile([P, P], f32, tag="maskT32")  # [k,q]: -1e30 if k > q
    make_lower_triangular(nc, maskT32[:], val=-1e30, diag=False)
    maskT = consts.tile([P, P], bf16, tag="maskT")
    nc.vector.tensor_copy(maskT[:], maskT32[:])
    ones32 = consts.tile([P, 1], f32, tag="ones32")
    nc.vector.memset(ones32[:], 1.0)
    # preload the ACT Exp table now (idle): the first Exp otherwise pays a
    # ~1.3us ACT_TABLE_LOAD inside phase B's serialized warm-up window
    expwarm = consts.tile([1, 1], bf16, tag="expwarm")
    nc.scalar.activation(expwarm[:], ones32[0:1, 0:1], Exp, scale=SCALE)

    # ---- persistent activations, split per 512-chunk so per-tile sems
    # never serialize phase B's early superblocks on phase A's tail ----
    # qp{pr}: [h_{2pr} r(32); h_{2pr} i(32); h_{2pr+1} r(32); h_{2pr+1} i(32)]
    def ptiles(tag, shape, dt):
        return [persist.tile(shape, dt, tag=f"{tag}{s}", name=f"{tag}{s}")
                for s in range(NSCH)]

    qp0_s = ptiles("qp0_", [P, SCH], bf16)
    qp1_s = ptiles("qp1_", [P, SCH], bf16)
    k2_s = ptiles("k2_", [P, SCH], bf16)             # [k_r; k_i] x2
    von_s = ptiles("von_", [P, 4, HD + 1], bf16)     # [k, kb%4, 65]
    vT_s = ptiles("vT_", [64, SCH], bf16)
    at0_s = at1_s = None
    if "N" in phases or "C" in phases:
        at0_s = ptiles("at0_", [P, SCH], bf16)       # heads 0,1
        at1_s = ptiles("at1_", [P, SCH], bf16)       # heads 2,3

    for s in range(NSCH):
        nc.vector.tensor_copy(von_s[s][:, :, HD:HD + 1],
                              ones32[:, None, :].to_broadcast((P, 4, 1)))

    # ================= Phase A: QKV projections + rope =================
    if "A" not in phases:
        return
    with (
        tc.tile_pool(name=pfx + "xtsb", bufs=8) as xt_pool,
        tc.tile_pool(name=pfx + "ropetmp", bufs=2) as rtmp_pool,
        tc.tile_pool(name=pfx + "qstage", bufs=2) as qst_pool,
        tc.tile_pool(name=pfx + "psA", bufs=2, space="PSUM") as psA,
        tc.tile_pool(name=pfx + "psAq", bufs=2, space="PSUM") as psAq,
    ):
        for sch in range(NSCH):
            s0 = sch * SCH
            ps_qr = psAq.tile([P, SCH], f32, tag="ps_qr")
            ps_qi = psAq.tile([P, SCH], f32, tag="ps_qi")
            ps_kv = psAq.tile([P, SCH], f32, tag="ps_kv")

            for db4 in range(NDBLK // 4):
                xt4 = xt_pool.tile([P, 4, SCH], bf16, tag="xt4")
                eng = nc.sync if db4 % 2 == 0 else nc.scalar
                eng.dma_start(xt4[:], xt_d[:, sch, 4 * db4:4 * db4 + 4, :])
                if "2" in phases:
                    continue
                for a in range(4):
                    db = 4 * db4 + a
                    st = db == 0
                    sp = db == NDBLK - 1
                    nc.tensor.matmul(ps_qr[:], wq_r[:, db, :], xt4[:, a, :],
                                     start=st, stop=sp)
                    nc.tensor.matmul(ps_qi[:], wq_i[:, db, :], xt4[:, a, :],
                                     start=st, stop=sp)
                    nc.tensor.matmul(ps_kv[:], wkvi[:, db, :], xt4[:, a, :],
                                     start=st, stop=sp)

            if sch == 0:
                # rope tables queued behind sch0's x tiles: x first
                nc.gpsimd.dma_start(cosT4[:], cos_d[:])
                nc.gpsimd.dma_start(sinT4[:], sin_d[:])
            elif sch == 2:
                # wo needed only by out_proj (~90us in): keep early DMA free
                nc.gpsimd.dma_start(wo_sb[:], wo_d[:])
            ssl = slice(s0, s0 + SCH)
            if "1" in phases or "2" in phases:
                continue
            if sch == NSCH - 1:
                # bounce the last chunk's psums to sbuf via fast ACT copies
                # so the psum pools release without waiting on the DVE rope
                qrc = qst_pool.tile([P, SCH], f32, tag="qrc")
                nc.scalar.copy(qrc[:], ps_qr[:])
                qic = qst_pool.tile([P, SCH], f32, tag="qic")
                nc.scalar.copy(qic[:], ps_qi[:])
                kvc_e = qst_pool.tile([32, SCH], f32, tag="kvc_e")
                nc.scalar.copy(kvc_e[:], ps_kv[0:32, :])
                kvc_o = qst_pool.tile([32, SCH], f32, tag="kvc_o")
                nc.scalar.copy(kvc_o[:], ps_kv[32:64, :])
                ps_qr, ps_qi = qrc, qic
                kv_e, kv_o = kvc_e[:], kvc_o[:]
            else:
                kv_e, kv_o = ps_kv[0:32, :], ps_kv[32:64, :]
            # ---- rope q (split layout: all 4 heads' r parts / i parts) ----
            qst_r = qst_pool.tile([P, SCH], bf16, tag="qst_r")
            qst_i = qst_pool.tile([P, SCH], bf16, tag="qst_i")
            ta = rtmp_pool.tile([P, SCH], f32, tag="ta")
            tb = rtmp_pool.tile([P, SCH], f32, tag="tb")
            tc2 = rtmp_pool.tile([P, SCH], f32, tag="tc2")
            td = rtmp_pool.tile([P, SCH], f32, tag="td")
            nc.vector.tensor_mul(ta[:], ps_qr[:], cosT4[:, ssl])
            nc.vector.tensor_mul(tb[:], ps_qi[:], sinT4[:, ssl])
            nc.vector.tensor_mul(tc2[:], ps_qr[:], sinT4[:, ssl])
            nc.vector.tensor_mul(td[:], ps_qi[:], cosT4[:, ssl])
            nc.vector.tensor_sub(qst_r[:], ta[:], tb[:])
            nc.vector.tensor_add(qst_i[:], tc2[:], td[:])
            # interleave into qp tiles: [h r; h i] per head (DMA)
            for h in range(HPC):
                qp = qp0_s[sch] if h < 2 else qp1_s[sch]
                b = 64 * (h % 2)
                nc.gpsimd.dma_start(qp[b:b + 32, :],
                                    qst_r[32 * h:32 * h + 32, :])
                nc.gpsimd.dma_start(qp[b + 32:b + 64, :],
                                    qst_i[32 * h:32 * h + 32, :])

            # ---- rope k (kv psum rows 0:64 = [k_e, k_o]) ----
            k2 = k2_s[sch]
            tka = rtmp_pool.tile([32, SCH], f32, tag="tka")
            tkb = rtmp_pool.tile([32, SCH], f32, tag="tkb")
            tkc = rtmp_pool.tile([32, SCH], f32, tag="tkc")
            tkd = rtmp_pool.tile([32, SCH], f32, tag="tkd")
            nc.vector.tensor_mul(tka[:], kv_e, cosT4[0:32, ssl])
            nc.vector.tensor_mul(tkb[:], kv_o, sinT4[0:32, ssl])
            nc.vector.tensor_mul(tkc[:], kv_e, sinT4[0:32, ssl])
            nc.vector.tensor_mul(tkd[:], kv_o, cosT4[0:32, ssl])
            nc.vector.tensor_sub(k2[0:32, :], tka[:], tkb[:])
            nc.vector.tensor_add(k2[32:64, :], tkc[:], tkd[:])
            # replicate [k_r; k_i] to rows 64:128 (DMA)
            nc.gpsimd.dma_start(k2[64:128, :], k2[0:64, :])
            # stash vT
            nc.scalar.copy(vT_s[sch][:], ps_kv[64:128, :])

            # ---- v natural ([k,65] with ones col) via PE transposes ----
            ps_v = psA.tile([P, 4, 64], bf16, tag="ps_v")
            for j in range(4):
                nc.tensor.transpose(ps_v[:, j, :],
                                    vT_s[sch][:, j * P:(j + 1) * P],
                                    ident[0:64, 0:64])
            nc.scalar.copy(von_s[sch][:, :, 0:HD], ps_v[:])
            if sch == 1:
                # warm up gpsimd's PartitionBroadcast ucode library mid-A
                # (the first broadcast after DMA-issue ops pays a ~7us
                # library reload; phase B's normalize must not)
                warm = rtmp_pool.tile([64, 64], f32, tag="warm")
                nc.gpsimd.partition_broadcast(warm[:], ones32[0:1, 0:1].to_broadcast((1, 64)))

    if "1" in phases or "2" in phases:
        return

    # ============ Phase B: attention (pair-split) + pipelined C ============
    if "B" not in phases:
        return
    do_exp = "E" in phases
    do_pv = "P" in phases
    do_norm = "N" in phases
    do_c = "C" in phases
    with (
        tc.tile_pool(name=pfx + "expt", bufs=4) as exp_pool,
        tc.tile_pool(name=pfx + "norm", bufs=4) as norm_pool,
        tc.tile_pool(name=pfx + "outcp", bufs=4) as outcp_pool,
        tc.tile_pool(name=pfx + "osb", bufs=4) as out_pool,
        tc.tile_pool(name=pfx + "psB", bufs=2, space="PSUM") as psB,
        tc.tile_pool(name=pfx + "psBo", bufs=2, space="PSUM") as psBo,
        tc.tile_pool(name=pfx + "psC", bufs=2, space="PSUM") as psC,
    ):
        def outproj_chunks(qsb):
            at0, at1 = at0_s[qsb], at1_s[qsb]
            tiles = [(j, dmc) for j in range(4) for dmc in range(4)]

            def mk(t):
                def chunk():
                    pair = tiles[t:t + 2]
                    pss = []
                    # both T0 matmuls first: they only need pr0's normalize
                    for i, (j, dmc) in enumerate(pair):
                        lsl = slice(j * P, (j + 1) * P)
                        dsl = slice(dmc * 512, (dmc + 1) * 512)
                        ps_o = psC.tile([P, 512], f32, tag="ps_o",
                                        name=f"pso{qsb}_{t}_{i}")
                        nc.tensor.matmul(ps_o[:], at0[:, lsl],
                                         wo_sb[:, 0, dsl],
                                         start=True, stop=False)
                        pss.append(ps_o)
                    for i, (j, dmc) in enumerate(pair):
                        lsl = slice(j * P, (j + 1) * P)
                        dsl = slice(dmc * 512, (dmc + 1) * 512)
                        nc.tensor.matmul(pss[i][:], at1[:, lsl],
                                         wo_sb[:, 1, dsl],
                                         start=False, stop=True)
                    for i, (j, dmc) in enumerate(pair):
                        sb = 4 * qsb + j
                        ssl = slice(sb * P, (sb + 1) * P)
                        dsl = slice(dmc * 512, (dmc + 1) * 512)
                        osb = out_pool.tile([P, 512], bf16, tag="osb",
                                            name=f"osb{qsb}_{t}_{i}")
                        nc.vector.tensor_copy(osb[:], pss[i][:])
                        eng = nc.sync if i == 0 else nc.scalar
                        eng.dma_start(out_d[ssl, dsl], osb[:])
                return chunk
            return [mk(t) for t in range(0, 16, 2)]

        def do_outproj(qsb):
            for c in outproj_chunks(qsb):
                c()

        for qsb in range(NQSB):
            q0 = qsb * QSB
            nkb = (q0 + QSB) // P
            # out_proj tile-pairs of the previous superblock, zipped between
            # attention blocks: wait-free PE work covering the exp round-trip
            op_pending = (outproj_chunks(qsb - 1)
                          if (do_c and do_norm and qsb >= 1) else [])
            blocks_total = 2 * nkb
            blocks_done = 0
            for pr in range(2):                     # head pairs (0,1), (2,3)
                qp = qp0_s[qsb] if pr == 0 else qp1_s[qsb]
                outps = [psBo.tile([HD + 1, QSB], f32, tag="outp",
                                   name=f"outp{qsb}_{pr}_{_m}") for _m in range(2)]
                def emit_pv(kb, expT, off):
                    sk, j = kb // 4, kb % 4
                    for m in range(2):
                        rhs = expT[:, m, off:] if do_exp else qp[:, off:]
                        nc.tensor.matmul(outps[m][:, off:],
                                         von_s[sk][:, j, :], rhs,
                                         start=(kb == 0), stop=(kb == nkb - 1),
                                         skip_group_check=True)

                pending = []
                for kb in range(nkb):
                    k0 = kb * P
                    sk, j = kb // 4, kb % 4
                    lks = slice(j * P, (j + 1) * P)
                    off = max(k0 - q0, 0)           # causal column trim
                    diag = k0 - q0 >= 0
                    scT = psB.tile([P, 2, QSB], f32, tag="scT")
                    for m in range(2):
                        rp = slice(64 * m, 64 * m + 64)
                        nc.tensor.matmul(scT[:, m, off:], k2_s[sk][rp, lks],
                                         qp[rp, off:],
                                         start=True, stop=True,
                                         tile_position=(64 * m, 0))
                    expT = exp_pool.tile([P, 2, QSB], bf16, tag="expT")
                    if diag:
                        nc.vector.tensor_add(
                            scT[:, :, off:off + P], scT[:, :, off:off + P],
                            maskT32[:, None, :].to_broadcast((P, 2, P)))
                    if do_exp:
                        nc.scalar.activation(expT[:, :, off:], scT[:, :, off:],
                                             Exp, scale=SCALE)
                    # software pipeline: PV trails the scores by 2 blocks,
                    # hiding the exp round-trip
                    pending.append((kb, expT, off))
                    blocks_done += 1
                    if do_pv and len(pending) > 1:
                        emit_pv(*pending.pop(0))
                if do_pv:
                    for args in pending:
                        emit_pv(*args)
                # normalize + place into attn_T
                if not do_norm:
                    continue
                dst = at0_s[qsb] if pr == 0 else at1_s[qsb]
                dens, recips, bcasts, ocps = [], [], [], []
                for m in range(2):
                    den = norm_pool.tile([1, QSB], f32, tag="den",
                                         name=f"den{qsb}_{pr}_{m}")
                    nc.vector.tensor_copy(den[:], outps[m][HD:HD + 1, :])
                    dens.append(den)
                for m in range(2):
                    recip = norm_pool.tile([1, QSB], f32, tag="recip",
                                           name=f"rc{qsb}_{pr}_{m}")
                    nc.vector.reciprocal_approx_fast(recip[:], dens[m][:])
                    recips.append(recip)
                for m in range(2):
                    bcast = norm_pool.tile([64, QSB], f32, tag="bcast",
                                           name=f"bc{qsb}_{pr}_{m}")
                    nc.gpsimd.partition_broadcast(bcast[:], recips[m][:])
                    bcasts.append(bcast)
                for m in range(2):
                    ocp = outcp_pool.tile([HD + 1, QSB], f32, tag="ocp",
                                          name=f"ocp{qsb}_{pr}_{m}")
                    nc.vector.tensor_copy(ocp[:], outps[m][:])
                    ocps.append(ocp)
                for m in range(2):
                    rsl = slice(64 * m, 64 * m + 64)
                    nc.vector.tensor_mul(dst[rsl, :], ocps[m][0:HD, :],
                                         bcasts[m][:])

            # drain any out_proj pairs not emitted during the zip
            for c in op_pending:
                c()
        if do_c and do_norm:
            do_outproj(NQSB - 1)
    tc.strict_bb_all_engine_barrier()


_NC_CACHE = {}


def _get_nc(reps=1, phases="ABEPNC"):
    key = (reps, phases)
    if key not in _NC_CACHE:
        _NC_CACHE[key] = _build_kernel(reps, phases)
    return _NC_CACHE[key]


def _sbufify(w2d):
    """[NDBLK*P, M] -> [P, NDBLK, M]: sbuf partition-major staging."""
    nblk = w2d.shape[0] // P
    return np.ascontiguousarray(
        w2d.reshape(nblk, P, w2d.shape[1]).transpose(1, 0, 2))


def _make_in_maps(x, wq, wk, wv, wo, freqs_cos, freqs_sin):
    x2 = np.asarray(x, dtype=np.float32).reshape(S, D)
    xT = _sbufify(np.ascontiguousarray(x2.T))            # [P, NDBLK, S]
    xT = np.ascontiguousarray(
        xT.reshape(P, NDBLK, NSCH, SCH).transpose(0, 2, 1, 3)).astype(BF16)
    cos = np.asarray(freqs_cos, dtype=np.float32)
    sin = np.asarray(freqs_sin, dtype=np.float32)
    cosT4 = np.ascontiguousarray(np.tile(cos.T, (HPC, 1)))
    sinT4 = np.ascontiguousarray(np.tile(sin.T, (HPC, 1)))
    wq = np.asarray(wq, dtype=np.float32)
    wk = np.asarray(wk, dtype=np.float32)
    wv = np.asarray(wv, dtype=np.float32)
    wo = np.asarray(wo, dtype=np.float32)

    in_maps = []
    for c in range(NCORES):
        wq_c = wq.reshape(D, NH, HD)[:, HPC * c:HPC * (c + 1), :]
        wq_r = np.ascontiguousarray(wq_c[:, :, 0::2].reshape(D, HPC * D2))
        wq_i = np.ascontiguousarray(wq_c[:, :, 1::2].reshape(D, HPC * D2))
        wk_c = wk.reshape(D, NKV, HD)[:, c, :]
        wv_c = wv.reshape(D, NKV, HD)[:, c, :]
        wkvi = np.ascontiguousarray(
            np.concatenate([wk_c[:, 0::2], wk_c[:, 1::2], wv_c], axis=1))
        wo_c = np.ascontiguousarray(
            wo.reshape(NH, HD, D)[HPC * c:HPC * (c + 1)].reshape(HPC * HD, D))
        in_maps.append({
            "xT": xT,
            "wq_r": _sbufify(wq_r).astype(BF16),
            "wq_i": _sbufify(wq_i).astype(BF16),
            "wkvi": _sbufify(wkvi).astype(BF16),
            "wo_c": _sbufify(wo_c).astype(BF16),
            "cosT4": cosT4, "sinT4": sinT4,
        })
    return in_maps


_last_in_maps = None


def kernel(x, wq, wk, wv, wo, freqs_cos, freqs_sin, mask):
    global _last_in_maps
    in_maps = _make_in_maps(x, wq, wk, wv, wo, freqs_cos, freqs_sin)
    _last_in_maps = in_maps
    nc = _get_nc()
    res = bass_utils.run_bass_kernel_spmd(nc, in_maps, core_ids=list(range(NCORES)))
    out = np.zeros((S, D), dtype=np.float64)
    for r in res.results:
        out += r["out"].astype(np.float64)
    return out.astype(np.float32).reshape(1, S, D)
